# revision 1
# baseline (speedup 1.0000x reference)
"""ARD RBF Gram matrix kernel for Trainium2 (8 NeuronCores, SPMD).

K[i, j] = exp(-0.5 * sum_d (x[i,d] - y[j,d])^2 / exp(logh[d]))

Strategy (per sharding hint): shard rows of x across the 8 cores; replicate
y and logh. Each core computes a [1024, 8192] tile of K.

Device-side algorithm per core:
  ih      = exp(-0.5 * logh)                     (ACT)
  xs      = x^T * ih      [d, i] layout          (DVE per-partition scale)
  ys_m2   = y^T * (-2 ih) [d, j] layout          (DVE per-partition scale)
  x2[i]   = sum_d xs^2        via ones-matmul    (DVE square + PE)
  y2[j]   = 0.25*sum_d ys_m2^2 via 0.25-matmul   (DVE square + PE)
  sq tile = sum_d xs^T.T @ ys_m2  (+ aug matmul adding x2[i] + y2[j])
            accumulated in PSUM (fp32r main matmuls, bf16 hi/lo aug)
  out     = exp(-0.5 * sq)                       (ACT, PSUM -> SBUF)
  DMA store to DRAM.

The host side only reshapes/transposes/shards numpy arrays; every floating
point operation happens on device.
"""

import json

import numpy as np

import concourse.bass as bass
import concourse.mybir as mybir
import concourse.tile as tile
from concourse.bass_utils import run_bass_kernel_spmd

N_CORES = 8
N, M, D = 8192, 8192, 512
NI = N // N_CORES  # rows of x per core (1024)
P = 128  # partitions
NCHUNK = D // P  # contraction chunks (4)
ITILES = NI // P  # i tiles per core (8)

F32 = mybir.dt.float32
F32R = mybir.dt.float32r
BF16 = mybir.dt.bfloat16
AF = mybir.ActivationFunctionType

# ---------------------------------------------------------------------------
# Workaround for this walrus build: only ONE sync-wait condition is allowed
# per instruction ("Too many sync wait commands"). Split excess on_wait
# entries onto preceding NoOps on the same engine (program order preserves
# semantics exactly).
# ---------------------------------------------------------------------------
_WAIT_LIMIT = 1


def _split_excess_waits(bir: dict, limit: int = _WAIT_LIMIT) -> dict:
    # Excess waits are moved onto preceding EventSemaphore instructions,
    # which this walrus accepts with up to TWO wait conditions (ordinary
    # instructions allow only one). Program order preserves semantics.
    counter = 0
    for fn in bir.get("functions", []):
        for bb in fn.get("blocks", []):
            new_insts = []
            for inst in bb.get("instructions", []):
                si = inst.get("sync_info")
                waits = si.get("on_wait") if si else None
                eng = inst.get("engine", "Unassigned")
                if waits and len(waits) > limit and eng != "Unassigned":
                    keep = len(waits) % 2  # odd count: last wait stays put
                    head = waits[: len(waits) - keep]
                    for i in range(0, len(head), 2):
                        counter += 1
                        new_insts.append(
                            {
                                "debug": inst.get("debug", 0),
                                "engine": eng,
                                "ins": [],
                                "outs": [],
                                "name": f"WS-{counter}-{inst['name']}",
                                "opcode": "EventSemaphore",
                                "sync_info": {
                                    "on_update": [],
                                    "on_wait": head[i : i + 2],
                                },
                            }
                        )
                    si["on_wait"] = waits[len(waits) - keep :]
                new_insts.append(inst)
            bb["instructions"] = new_insts
    return bir


def _patch_nc(nc):
    orig = nc.to_json_bytes

    def patched() -> bytes:
        return json.dumps(_split_excess_waits(json.loads(orig()))).encode()

    nc.to_json_bytes = patched
    return nc


# ---------------------------------------------------------------------------
# Device program (identical on all 8 cores; only DRAM contents differ)
# ---------------------------------------------------------------------------


W = 1024  # max y strip width (pipelining granule)
STRIP_PLAN = [(s0, W) for s0 in range(0, M, W)]
NSTRIP = len(STRIP_PLAN)

# "f32r": fp32r main matmuls (near-fp32 accuracy).
# "bf16": bf16 main matmuls (faster weight loads; the Gram cross-term gets
#         bf16 rounding, norms/aug stay fp32-grade).
MAIN_DTYPE = "f32r"


def _build_nc():
    nc = bass.Bass()

    xt = nc.dram_tensor("xt", [D, NI], F32, kind="ExternalInput")
    yt = nc.dram_tensor("yt", [D, M], F32, kind="ExternalInput")
    lh = nc.dram_tensor("lh", [NCHUNK, P], F32, kind="ExternalInput")
    out = nc.dram_tensor("out", [NI, M], F32, kind="ExternalOutput")

    yt_r = yt.rearrange("(c d) j -> d c j", d=P)

    f32r_mode = MAIN_DTYPE == "f32r"

    def mm_view(ap):
        # view an f32 AP as fp32r in f32r mode (same bytes)
        return ap.bitcast(F32R) if f32r_mode else ap

    with tile.TileContext(nc) as tc:
        with (
            tc.tile_pool(name="singles", bufs=1) as singles,
            tc.tile_pool(name="ystrips", bufs=4) as ystrips,
            tc.tile_pool(name="sqp", bufs=2) as sqp,
            tc.tile_pool(name="stp", bufs=2) as stp,
            tc.tile_pool(name="accp", bufs=2, space="PSUM") as accp,
            tc.tile_pool(name="outp", bufs=3) as outp,
            tc.tile_pool(name="mainps", bufs=3, space="PSUM") as mainps,
        ):
            # persistent SBUF tensors
            xs = singles.tile([P, NCHUNK, NI], F32)  # raw x^T, then ih*x^T
            aug_l = singles.tile([4, NI], BF16)  # rows: x2_hi, x2_lo, 1, 1
            aug_r = singles.tile([4, M], BF16)  # rows: 1, 1, y2_hi, y2_lo
            lhs = singles.tile([P, NCHUNK], F32)
            ih = singles.tile([P, NCHUNK], F32)
            ihm2 = singles.tile([P, NCHUNK], F32)
            ihsq = singles.tile([P, NCHUNK], F32)  # ih^2, reduce-matmul lhsT
            if f32r_mode:
                xs_mm = xs  # scaled in place (rounded via fp32r view)
            else:
                xs_mm = singles.tile([P, NCHUNK, NI], BF16)

            # In f32r mode loads are tagged fp32r purely for the BIR
            # verifier: the DVE scale rewrites (and genuinely rounds) every
            # element before any fp32r matmul reads these tensors.
            nc.sync.dma_start(out=lhs, in_=lh.rearrange("c d -> d c"))
            xt_r = xt.rearrange("(c d) i -> d c i", d=P)
            for c in range(NCHUNK):
                # per-chunk loads: the scale of chunk c starts as soon as
                # its quarter of the data lands, not after the full tensor
                nc.sync.dma_start(
                    out=mm_view(xs[:, c, :]), in_=mm_view(xt_r[:, c, :])
                )

            # ih = exp(-0.5*logh); ihm2 = -2*ih; ihsq = ih^2
            nc.scalar.activation(ih, lhs, AF.Exp, scale=-0.5)
            nc.vector.tensor_scalar_mul(ihm2, ih, -2.0)
            nc.vector.tensor_mul(mm_view(ihsq), ih, ih)
            if not f32r_mode:
                # reduce-matmul lhsT vectors (squares are computed from the
                # already-scaled bf16 data in this mode)
                vec_one = singles.tile([P, 1], BF16)
                vec_quarter = singles.tile([P, 1], BF16)
                nc.vector.memset(vec_one, 1.0)
                nc.vector.memset(vec_quarter, 0.25)
            # data rows of aug_l/aug_r are DMA-overwritten below (engine APs
            # must start at a 32-aligned partition; DMA APs are unrestricted).
            # GpSimd is idle and keeps these off the DVE critical chain.
            nc.gpsimd.memset(aug_l, 1.0)
            nc.gpsimd.memset(aug_r, 1.0)

            def reduce_sq(src, w, dst, dst_rows, dst0, pfx, lhsT_vec=None):
                # dst[dst_rows, dst0:dst0+w] = (hi, lo) bf16 split of
                # sum_d lhsT_vec * src^2.  In f32r mode src is raw f32 and
                # lhsT_vec defaults to ihsq; in bf16 mode src is the scaled
                # bf16 data and the caller passes a constant vector.
                nj = w // 512
                sq_dt = F32 if f32r_mode else BF16
                accs = [
                    accp.tile([1, 512], F32, tag="acc", name=f"{pfx}a{k}")
                    for k in range(nj)
                ]
                for c in range(NCHUNK):
                    sq = sqp.tile([P, w], sq_dt, tag="sq", name=f"{pfx}sq{c}")
                    nc.vector.tensor_mul(mm_view(sq), src[:, c, :], src[:, c, :])
                    vec = lhsT_vec if lhsT_vec is not None else ihsq[:, c : c + 1]
                    for js in range(nj):
                        nc.tensor.matmul(
                            accs[js],
                            mm_view(vec),
                            mm_view(sq[:, js * 512 : (js + 1) * 512]),
                            start=(c == 0),
                            stop=(c == NCHUNK - 1),
                        )
                st_h = stp.tile([1, w], BF16, tag="sth", name=f"{pfx}h")
                st_l = stp.tile([1, w], BF16, tag="stl", name=f"{pfx}l")
                for js in range(nj):
                    sl = slice(js * 512, (js + 1) * 512)
                    nc.vector.tensor_copy(st_h[:, sl], accs[js])
                    nc.vector.tensor_sub(st_l[:, sl], accs[js], st_h[:, sl])
                nc.sync.dma_start(
                    out=dst[dst_rows[0] : dst_rows[0] + 1, dst0 : dst0 + w],
                    in_=st_h,
                )
                nc.sync.dma_start(
                    out=dst[dst_rows[1] : dst_rows[1] + 1, dst0 : dst0 + w],
                    in_=st_l,
                )

            # ---- x prep (once) ----
            if f32r_mode:
                reduce_sq(xs, NI, aug_l, (0, 1), 0, "x")
            for c in range(NCHUNK):
                nc.vector.tensor_scalar_mul(
                    mm_view(xs_mm[:, c, :]), xs[:, c, :], ih[:, c : c + 1]
                )

            # ---- y strips: load -> y2 reduce -> scale; interleaved with
            # the previous strip's main-loop work so the PE never starves
            strip_tiles = {}

            def prep_strip(s):
                s0, w = STRIP_PLAN[s]
                yst = ystrips.tile(
                    [P, NCHUNK, w], F32, tag="ystrip", name=f"ystrip{s}"
                )
                for c in range(NCHUNK):
                    nc.sync.dma_start(
                        out=mm_view(yst[:, c, :]),
                        in_=mm_view(yt_r[:, c, s0 : s0 + w]),
                    )
                if f32r_mode:
                    reduce_sq(yst, w, aug_r, (2, 3), s0, f"y{s}")
                    ymm = yst
                else:
                    ymm = ystrips.tile(
                        [P, NCHUNK, w], BF16, tag="ybf", name=f"ybf{s}"
                    )
                strip_tiles[s] = ymm
                for c in range(NCHUNK):
                    nc.vector.tensor_scalar_mul(
                        mm_view(ymm[:, c, :]),
                        yst[:, c, :],
                        ihm2[:, c : c + 1],
                    )
                if not f32r_mode:
                    # y2 = 0.25 * sum (-2 ih y)^2 from the scaled bf16 copy
                    reduce_sq(
                        ymm, w, aug_r, (2, 3), s0, f"y{s}",
                        lhsT_vec=vec_quarter,
                    )

            def main_strip(s):
                s0, w = STRIP_PLAN[s]
                ymm = strip_tiles[s]
                for it in range(ITILES):
                    lhsT_slices = [
                        mm_view(xs_mm[:, c, it * P : (it + 1) * P])
                        for c in range(NCHUNK)
                    ]
                    aug_lhsT = aug_l[:, it * P : (it + 1) * P]
                    ps = mainps.tile([P, w], F32, tag="ps", name=f"ps{s}_{it}")
                    for c in range(NCHUNK):
                        for js in range(w // 512):
                            nc.tensor.matmul(
                                ps[:, js * 512 : (js + 1) * 512],
                                lhsT_slices[c],
                                mm_view(ymm[:, c, js * 512 : (js + 1) * 512]),
                                start=(c == 0),
                                stop=False,
                            )
                    for js in range(w // 512):
                        nc.tensor.matmul(
                            ps[:, js * 512 : (js + 1) * 512],
                            aug_lhsT,
                            aug_r[:, s0 + js * 512 : s0 + (js + 1) * 512],
                            start=False,
                            stop=True,
                        )
                    ot = outp.tile([P, w], F32, tag="ot", name=f"ot{s}_{it}")
                    nc.scalar.activation(ot, ps, AF.Exp, scale=-0.5)
                    nc.sync.dma_start(
                        out=out[it * P : (it + 1) * P, s0 : s0 + w],
                        in_=ot,
                    )

            prep_strip(0)
            if not f32r_mode:
                # x2 reduction emitted after strip 0's scale ops so the DVE
                # work the first main matmuls wait on (x-scale + strip-0
                # scale) runs first; x2 is only needed by the aug matmuls.
                reduce_sq(xs_mm, NI, aug_l, (0, 1), 0, "x", lhsT_vec=vec_one)
            for s in range(NSTRIP):
                if s + 1 < NSTRIP:
                    prep_strip(s + 1)
                main_strip(s)

    return _patch_nc(nc)


_NC_CACHE = None

# test.py hooks: set _TRACE to capture a profile; results object stored here.
_TRACE = False
_TRACE_KWARGS = {}
LAST_RESULTS = None


def kernel(x, y, logh):
    global _NC_CACHE, LAST_RESULTS
    x = np.ascontiguousarray(np.asarray(x, dtype=np.float32))
    y = np.ascontiguousarray(np.asarray(y, dtype=np.float32))
    logh = np.ascontiguousarray(np.asarray(logh, dtype=np.float32))
    assert x.shape == (N, D) and y.shape == (M, D) and logh.shape == (D,)

    if _NC_CACHE is None:
        _NC_CACHE = _build_nc()
    nc = _NC_CACHE

    ytp = np.ascontiguousarray(y.T)  # [D, M]
    lh = np.ascontiguousarray(logh.reshape(NCHUNK, P))
    in_maps = []
    for c in range(N_CORES):
        xtc = np.ascontiguousarray(x[c * NI : (c + 1) * NI, :].T)  # [D, NI]
        in_maps.append({"xt": xtc, "yt": ytp, "lh": lh})

    res = run_bass_kernel_spmd(
        nc,
        in_maps,
        core_ids=list(range(N_CORES)),
        trace=_TRACE,
        **_TRACE_KWARGS,
    )
    LAST_RESULTS = res
    return np.concatenate(
        [res.results[c]["out"] for c in range(N_CORES)], axis=0
    )

